# revision 1
# baseline (speedup 1.0000x reference)
"""2-layer GCN (GCNConv -> ReLU -> GCNConv -> log_softmax) on 8 TRN2 NeuronCores.

v2: aggregation via bulk dma_gather (InstDMAGatherAnt) instead of per-slot
indirect DMAs. One instruction gathers thousands of 64B rows (elem payload 16
f32, source row stride 256B), smashing the 128-rows-per-994ns SWDGE limit of
indirect_dma_start.

- Nodes sharded by destination across 8 cores; within a shard, nodes are
  degree-sorted so 128-node blocks have near-uniform in-degree.
- Self-loops are appended as ordinary edges (table rows are pre-scaled by
  dis = deg^-1/2, outputs post-scaled by dis).
- int16 gather indices only address <32768 rows, so the table is split into
  4 quarter views; each block's slots are grouped by source quarter (padded
  per (block, quarter) to the cross-core max), and each (chunk-of-blocks,
  quarter) is one dma_gather. Pads point at a per-quarter zero row.
- Per block: 4 strided-view reduce_sums (one per quarter) + 3 adds -> acc.
- Both layers aggregate in 16-feature space; layer-2's linear transform is
  applied after aggregation. Tables exchanged with AllGather (rows padded to
  64 f32 for the 256B-stride requirement).
"""

import numpy as np
import concourse.bacc as bacc
import concourse.bass as bass
import concourse.mybir as mybir
from concourse.tile import TileContext
from concourse.masks import make_identity
from concourse.bass_utils import run_bass_kernel_spmd

F32 = mybir.dt.float32
I16 = mybir.dt.int16

N_NODES = 100000
N_FEAT = 500
HID = 16
N_CLS = 40
N_CORES = 8

QSHIFT = 15  # quarter = row >> 15 (32768 rows per quarter view)
QSIZE = 1 << QSHIFT

# gather mode: "B" = raw 64B payload / 256B stride; "A" = documented 256B
GATHER_MODE = "B"
W_TAB = 64  # table row stride in f32 (256B)
G_CHUNK = 8 if GATHER_MODE == "B" else 3
MAX_GCOLS = 32  # max 128-row columns per dma_gather (4096 descriptors,
# multi-packet; single_packet=True only below the 1024-desc packet cap)


class _Cfg:
    def __init__(self, n_nodes, fin, hid, ncls, n_cores=8):
        self.N = n_nodes
        self.FIN = fin
        self.H = hid
        self.C = ncls
        self.NC = n_cores
        self.SHARD = n_nodes // n_cores
        self.SHARD_PAD = ((self.SHARD + 127) // 128) * 128
        self.NB = self.SHARD_PAD // 128
        self.TROWS = n_cores * self.SHARD_PAD
        assert self.SHARD_PAD > self.SHARD, "need at least one pad row"
        self.NQ = (self.TROWS + QSIZE - 1) // QSIZE
        assert self.NQ == 4
        self.KC = max(1, (fin + 127) // 128)
        assert fin % self.KC == 0
        self.CHUNK = fin // self.KC
        self.XS = 16  # blocks per xT supertile


def _zero_rows(cfg):
    """Per-quarter zero row (quarter-local index of a known-zero pad row)."""
    zq = []
    for q in range(cfg.NQ):
        lo, hi = q * QSIZE, min((q + 1) * QSIZE, cfg.TROWS)
        found = None
        for c in range(cfg.NC):
            pr = c * cfg.SHARD_PAD + cfg.SHARD  # first pad row of core c
            if lo <= pr < hi:
                found = pr - lo
                break
        assert found is not None, f"no zero row in quarter {q}"
        zq.append(found)
    return zq


def _preprocess(x, edge_index, cfg):
    N, NC, SP, NB = cfg.N, cfg.NC, cfg.SHARD_PAD, cfg.NB
    src = np.asarray(edge_index[0], dtype=np.int64)
    dst = np.asarray(edge_index[1], dtype=np.int64)
    deg = np.bincount(dst, minlength=N).astype(np.int64) + 1
    dis = (1.0 / np.sqrt(deg.astype(np.float64))).astype(np.float32)

    pid = np.empty(N, dtype=np.int64)
    perm_list = []
    for c in range(NC):
        nodes = np.arange(c * cfg.SHARD, (c + 1) * cfg.SHARD)
        order = np.argsort(-deg[nodes], kind="stable")
        local = nodes[order]
        perm_list.append(local)
        pid[local] = c * SP + np.arange(cfg.SHARD)

    # append self-loops as ordinary edges
    loop = np.arange(N, dtype=np.int64)
    src_f = np.concatenate([src, loop])
    dst_f = np.concatenate([dst, loop])
    src_pid = pid[src_f]
    dst_pid = pid[dst_f]

    core_of = dst_pid // SP
    q_of = (src_pid >> QSHIFT).astype(np.int64)

    # per-core edge buckets sorted by (quarter, dst_local); slot ranks within
    cnt_qb = np.zeros((NC, cfg.NQ, SP), dtype=np.int64)
    buckets = []
    for c in range(NC):
        m = core_of == c
        dl = dst_pid[m] - c * SP
        sp_ = src_pid[m]
        qq = q_of[m]
        key = qq * SP + dl
        o = np.argsort(key, kind="stable")
        dl, sp_, qq, key = dl[o], sp_[o], qq[o], key[o]
        cnt = np.bincount(key, minlength=cfg.NQ * SP)
        starts = np.concatenate([[0], np.cumsum(cnt)])[:-1]
        s_arr = np.arange(dl.size) - starts[key]
        buckets.append((dl, sp_, qq, s_arr))
        cnt_qb[c] = cnt.reshape(cfg.NQ, SP)

    # S_qb = max over cores & nodes-in-block of per-(node, quarter) count
    S_qb = cnt_qb.reshape(NC, cfg.NQ, NB, 128).max(axis=(0, 3))  # [NQ, NB]

    # chunk layout
    chunks = []  # list of dicts
    col = 0
    for b0 in range(0, NB, G_CHUNK):
        b1 = min(b0 + G_CHUNK, NB)
        qranges = []  # per q: (colstart, ncols)
        bounds = {}  # (q, b) -> colstart
        c0 = col
        for q in range(cfg.NQ):
            qs = col
            for b in range(b0, b1):
                bounds[(q, b)] = col
                col += int(S_qb[q, b])
            qranges.append((qs, col - qs))
        chunks.append(dict(b0=b0, b1=b1, c0=c0, ncols=col - c0,
                           qranges=qranges, bounds=bounds))
    TOTC = col

    zq = _zero_rows(cfg)
    # per-core column grid of quarter-local int16 indices
    idx16 = np.empty((NC, 128, TOTC), dtype=np.int16)
    colbase = np.zeros((cfg.NQ, NB), dtype=np.int64)
    for ch in chunks:
        for q in range(cfg.NQ):
            for b in range(ch["b0"], ch["b1"]):
                colbase[q, b] = ch["bounds"][(q, b)]
    # fill pads per quarter-range
    for ch in chunks:
        for q in range(cfg.NQ):
            qs, qn = ch["qranges"][q]
            idx16[:, :, qs:qs + qn] = zq[q]
    for c in range(NC):
        dl, sp_, qq, s_arr = buckets[c]
        b_arr = dl // 128
        p_arr = dl % 128
        t_arr = colbase[qq, b_arr] + s_arr
        idx16[c, p_arr, t_arr] = (sp_ - (qq << QSHIFT)).astype(np.int16)

    # wrap: flat i = col*128 + p -> tile[(i%16), i//16], replicated 8x.
    # Per chunk: flat = grid[:, c0:c1].T.flatten()
    idx_wrapped = np.empty((NC, 128, TOTC * 8), dtype=np.int16)
    for c in range(NC):
        pieces = []
        for ch in chunks:
            flat = idx16[c, :, ch["c0"]:ch["c0"] + ch["ncols"]].T.reshape(-1)
            wrap = flat.reshape(-1, 16).T  # [16, ncols*8]
            pieces.append(np.tile(wrap, (8, 1)))
        idx_wrapped[c] = np.concatenate(pieces, axis=1)

    dis_pm = np.zeros((NC, 128, NB), dtype=np.float32)
    for c in range(NC):
        d = np.zeros(SP, dtype=np.float32)
        d[: cfg.SHARD] = dis[perm_list[c]]
        dis_pm[c] = d.reshape(NB, 128).T

    xT = np.zeros((NC, cfg.FIN, SP), dtype=np.float32)
    for c in range(NC):
        xc = np.zeros((SP, cfg.FIN), dtype=np.float32)
        xc[: cfg.SHARD] = x[perm_list[c]]
        xT[c] = np.ascontiguousarray(xc.T)

    return dict(idx_wrapped=idx_wrapped, dis_pm=dis_pm, xT=xT,
                chunks=chunks, TOTC=TOTC, S_qb=S_qb, perm_list=perm_list)


def _raw_dma_gather(nc, out_ap, in_ap, idxs_ap, num_idxs, elem_size,
                    elem_step, single_packet=True):
    gp = nc.gpsimd
    stride_bytes = elem_step * mybir.dt.size(in_ap.dtype)
    assert stride_bytes % 256 == 0
    _in_ap = gp.lower_ap_dma(in_ap, for_custom_bir_dma=True)
    _idxs_ap = gp.lower_ap(idxs_ap)
    _out_ap = gp.lower_ap(out_ap)
    return gp.add_instruction(
        mybir.InstDMAGatherAnt(
            name=nc.get_next_instruction_name(),
            ins=[*_in_ap, _idxs_ap,
                 gp.lower_val_access(gp.to_reg(num_idxs))],
            outs=[_out_ap],
            transpose=False,
            num_idxs=num_idxs,
            elem_size=elem_size,
            stride_bytes_256=stride_bytes // 256,
            gen_mode=0,
            single_packet=single_packet,
            queue_num=0,
            sbuf_tokens_per_rank=0,
            sbuf_free_dim_per_rank=0,
            sbuf_free_dim_pad_per_rank=0,
            sbuf_byte_offset=0,
        ))


def _build_kernel(cfg, pre):
    nc = bacc.Bacc("TRN2")
    FIN, H, C, SP, NB = cfg.FIN, cfg.H, cfg.C, cfg.SHARD_PAD, cfg.NB
    KC, CH = cfg.KC, cfg.CHUNK
    chunks, TOTC = pre["chunks"], pre["TOTC"]
    GW = H if GATHER_MODE == "B" else W_TAB

    xT = nc.dram_tensor("xT", [FIN, SP], F32, kind="ExternalInput")
    w1 = nc.dram_tensor("w1", [FIN, H], F32, kind="ExternalInput")
    b1r = nc.dram_tensor("b1r", [128, H], F32, kind="ExternalInput")
    w2 = nc.dram_tensor("w2", [H, C], F32, kind="ExternalInput")
    b2r = nc.dram_tensor("b2r", [128, C], F32, kind="ExternalInput")
    dis_d = nc.dram_tensor("dis", [128, NB], F32, kind="ExternalInput")
    idx_d = nc.dram_tensor("idx", [128, TOTC * 8], I16, kind="ExternalInput")
    out_d = nc.dram_tensor("out", [SP, C], F32, kind="ExternalOutput")

    h1_own = nc.dram_tensor("h1_own", [SP, W_TAB], F32)
    y2_own = nc.dram_tensor("y2_own", [SP, W_TAB], F32)
    table1 = nc.dram_tensor("table1", [cfg.TROWS, W_TAB], F32,
                            addr_space="Shared")
    table2 = nc.dram_tensor("table2", [cfg.TROWS, W_TAB], F32,
                            addr_space="Shared")

    groups = [list(range(cfg.NC))]

    with TileContext(nc) as tc:
        with tc.tile_pool(name="const", bufs=1) as constp, \
             tc.tile_pool(name="xsup", bufs=2) as xsupp, \
             tc.tile_pool(name="ps_h", bufs=4, space="PSUM") as ps_h, \
             tc.tile_pool(name="ps_t", bufs=2, space="PSUM") as ps_t, \
             tc.tile_pool(name="ps_o", bufs=2, space="PSUM") as ps_o, \
             tc.tile_pool(name="hsb", bufs=4) as hsbp, \
             tc.tile_pool(name="g", bufs=2) as gp_pool, \
             tc.tile_pool(name="gi", bufs=2) as gip, \
             tc.tile_pool(name="acc", bufs=4) as accp, \
             tc.tile_pool(name="ep", bufs=4) as epp:

            w1t = constp.tile([CH, KC, H], F32)
            for k in range(KC):
                nc.sync.dma_start(out=w1t[:, k, :],
                                  in_=w1[k * CH:(k + 1) * CH, :])
            w2t = constp.tile([H, C], F32)
            nc.sync.dma_start(out=w2t[:], in_=w2[:])
            b1t = constp.tile([128, H], F32)
            nc.sync.dma_start(out=b1t[:], in_=b1r[:])
            b2t = constp.tile([128, C], F32)
            nc.sync.dma_start(out=b2t[:], in_=b2r[:])
            dis_t = constp.tile([128, NB], F32)
            nc.sync.dma_start(out=dis_t[:], in_=dis_d[:])
            ident = constp.tile([128, 128], F32)
            make_identity(nc, ident[:])

            # Phase A: h1_own = dis * (x @ W1)
            nxs = (NB + cfg.XS - 1) // cfg.XS
            for si in range(nxs):
                b_lo = si * cfg.XS
                b_hi = min(NB, b_lo + cfg.XS)
                w = (b_hi - b_lo) * 128
                xts = xsupp.tile([CH, KC, cfg.XS * 128], F32, tag="xts")
                for k in range(KC):
                    nc.sync.dma_start(
                        out=xts[:, k, :w],
                        in_=xT[k * CH:(k + 1) * CH, b_lo * 128:b_hi * 128])
                for b in range(b_lo, b_hi):
                    j = (b - b_lo) * 128
                    ph = ps_h.tile([128, H], F32, tag="ph")
                    for k in range(KC):
                        nc.tensor.matmul(
                            out=ph[:], lhsT=xts[:, k, j:j + 128],
                            rhs=w1t[:, k, :],
                            start=(k == 0), stop=(k == KC - 1))
                    hsb = hsbp.tile([128, H], F32, tag="hsb")
                    nc.scalar.mul(out=hsb[:], in_=ph[:], mul=dis_t[:, b:b + 1])
                    nc.sync.dma_start(
                        out=h1_own[b * 128:(b + 1) * 128, 0:H], in_=hsb[:])

            nc.gpsimd.collective_compute(
                "AllGather", mybir.AluOpType.bypass, replica_groups=groups,
                ins=[h1_own[:, :]], outs=[table1[:, :]])

            MAXC = max(ch["ncols"] for ch in chunks)

            def aggregate(table, post_block):
                for ch in chunks:
                    ncols = ch["ncols"]
                    g = gp_pool.tile([128, ncols, GW], F32, tag="g",
                                     padded_shape=[128, MAXC, GW])
                    gi = gip.tile([128, ncols * 8], I16, tag="gi",
                                  padded_shape=[128, MAXC * 8])
                    nc.sync.dma_start(
                        out=gi[:, :ncols * 8],
                        in_=idx_d[:, ch["c0"] * 8:(ch["c0"] + ncols) * 8])
                    for q in range(cfg.NQ):
                        qs, qn = ch["qranges"][q]
                        if qn == 0:
                            continue
                        lo = q * QSIZE
                        hi = min(lo + QSIZE, cfg.TROWS)
                        o0 = qs - ch["c0"]
                        # split: SWDGE descriptor scratch holds <16384
                        # descriptors per instruction; stay at <=8192
                        for p0 in range(0, qn, MAX_GCOLS):
                            pn = min(MAX_GCOLS, qn - p0)
                            oo = o0 + p0
                            if GATHER_MODE == "B":
                                _raw_dma_gather(
                                    nc, g[:, oo:oo + pn, :],
                                    table[lo:hi, 0:H],
                                    gi[:, oo * 8:(oo + pn) * 8],
                                    pn * 128, H, W_TAB,
                                    single_packet=(pn * 128 <= 768))
                            else:
                                nc.gpsimd.dma_gather(
                                    out_ap=g[:, oo:oo + pn, :],
                                    in_ap=table[lo:hi, :],
                                    idxs_ap=gi[:, oo * 8:(oo + pn) * 8],
                                    num_idxs=pn * 128,
                                    num_idxs_reg=pn * 128,
                                    elem_size=W_TAB)
                    for b in range(ch["b0"], ch["b1"]):
                        acc = accp.tile([128, H], F32, tag="acc")
                        first = True
                        for q in range(cfg.NQ):
                            S = int(pre["S_qb"][q, b])
                            if S == 0:
                                continue
                            o = ch["bounds"][(q, b)] - ch["c0"]
                            view = g[:, o:o + S, 0:H].transpose([0, 2, 1])
                            if first:
                                nc.vector.reduce_sum(
                                    out=acc[:], in_=view,
                                    axis=mybir.AxisListType.X)
                                first = False
                            else:
                                pq = epp.tile([128, H], F32, tag="pq")
                                nc.vector.reduce_sum(
                                    out=pq[:], in_=view,
                                    axis=mybir.AxisListType.X)
                                nc.vector.tensor_add(out=acc[:], in0=acc[:],
                                                     in1=pq[:])
                        post_block(b, acc)

            def post1(b, acc):
                dis_col = dis_t[:, b:b + 1]
                v = epp.tile([128, H], F32, tag="v1")
                nc.vector.tensor_scalar_mul(out=v[:], in0=acc[:],
                                            scalar1=dis_col)
                nc.vector.tensor_add(out=v[:], in0=v[:], in1=b1t[:])
                r = epp.tile([128, H], F32, tag="r1")
                nc.scalar.activation(out=r[:], in_=v[:],
                                     func=mybir.ActivationFunctionType.Relu)
                y = epp.tile([128, H], F32, tag="y1")
                nc.vector.tensor_scalar_mul(out=y[:], in0=r[:],
                                            scalar1=dis_col)
                nc.sync.dma_start(out=y2_own[b * 128:(b + 1) * 128, 0:H],
                                  in_=y[:])

            aggregate(table1, post1)

            nc.gpsimd.collective_compute(
                "AllGather", mybir.AluOpType.bypass, replica_groups=groups,
                ins=[y2_own[:, :]], outs=[table2[:, :]])

            def post2(b, acc):
                dis_col = dis_t[:, b:b + 1]
                a = epp.tile([128, H], F32, tag="a2")
                nc.vector.tensor_scalar_mul(out=a[:], in0=acc[:],
                                            scalar1=dis_col)
                pt = ps_t.tile([H, 128], F32, tag="pt")
                nc.tensor.transpose(out=pt[:], in_=a[:], identity=ident[:])
                at = epp.tile([H, 128], F32, tag="at")
                nc.vector.tensor_copy(out=at[:], in_=pt[:])
                po = ps_o.tile([128, C], F32, tag="po")
                nc.tensor.matmul(out=po[:], lhsT=at[:], rhs=w2t[:],
                                 start=True, stop=True)
                o1 = epp.tile([128, C], F32, tag="o1")
                nc.vector.tensor_add(out=o1[:], in0=po[:], in1=b2t[:])
                mx = epp.tile([128, 1], F32, tag="mx")
                nc.vector.reduce_max(out=mx[:], in_=o1[:],
                                     axis=mybir.AxisListType.X)
                tt = epp.tile([128, C], F32, tag="tt")
                nc.vector.tensor_scalar(out=tt[:], in0=o1[:], scalar1=mx[:],
                                        scalar2=None,
                                        op0=mybir.AluOpType.subtract)
                ex = epp.tile([128, C], F32, tag="ex")
                nc.scalar.activation(out=ex[:], in_=tt[:],
                                     func=mybir.ActivationFunctionType.Exp)
                sm = epp.tile([128, 1], F32, tag="sm")
                nc.vector.reduce_sum(out=sm[:], in_=ex[:],
                                     axis=mybir.AxisListType.X)
                ls = epp.tile([128, 1], F32, tag="ls")
                nc.scalar.activation(out=ls[:], in_=sm[:],
                                     func=mybir.ActivationFunctionType.Ln)
                fin = epp.tile([128, C], F32, tag="fin")
                nc.vector.tensor_scalar(out=fin[:], in0=tt[:], scalar1=ls[:],
                                        scalar2=None,
                                        op0=mybir.AluOpType.subtract)
                nc.sync.dma_start(out=out_d[b * 128:(b + 1) * 128, :],
                                  in_=fin[:])

            aggregate(table2, post2)

    nc.compile()
    return nc


def kernel(x, edge_index, W1, b1, W2, b2):
    x = np.asarray(x)
    edge_index = np.asarray(edge_index)
    W1 = np.asarray(W1, np.float32)
    b1 = np.asarray(b1, np.float32)
    W2 = np.asarray(W2, np.float32)
    b2 = np.asarray(b2, np.float32)

    cfg = _Cfg(x.shape[0], x.shape[1], W1.shape[1], W2.shape[1], N_CORES)
    pre = _preprocess(x, edge_index, cfg)
    nc = _build_kernel(cfg, pre)

    b1r = np.broadcast_to(b1, (128, cfg.H)).copy()
    b2r = np.broadcast_to(b2, (128, cfg.C)).copy()
    in_maps = []
    for c in range(cfg.NC):
        in_maps.append({
            "xT": pre["xT"][c],
            "w1": W1,
            "b1r": b1r,
            "w2": W2,
            "b2r": b2r,
            "dis": pre["dis_pm"][c],
            "idx": pre["idx_wrapped"][c],
        })
    r = run_bass_kernel_spmd(nc, in_maps, list(range(cfg.NC)))
    out = np.empty((cfg.N, cfg.C), dtype=np.float32)
    for c in range(cfg.NC):
        out[pre["perm_list"][c]] = r.results[c]["out"][: cfg.SHARD]
    return out



# revision 21
# speedup vs baseline: 3.6773x; 3.6773x over previous
"""2-layer GCN (GCNConv -> ReLU -> GCNConv -> log_softmax) on 8 TRN2 NeuronCores.

v3: aggregation via gpsimd ap_gather from an SBUF-resident feature-major table
plus a cumsum/segment-difference reduction, replacing v2's InstDMAGatherAnt
(whose Q7 descriptor generation at ~7ns/descriptor dominated the runtime).

- Nodes sharded by destination: core c owns dsts [12500c, 12500(c+1)).
- Table layout: [128 partitions, 12544] f32 where partition (w, f) holds
  feature f of source-shard w's nodes, rows pre-scaled by dis = deg^-1/2.
  Built per layer by AllGather of each core's own [16, 12544] block.
- Gather: ap_gather (Q7 ucode, ~27ns/idx/core, 8 windows in parallel) pulls
  edge messages in dst-sorted order per (dst-chunk, window). Segment sums via
  tensor_tensor_scan cumsum + ap_gather of per-dst end positions + adjacent
  difference; padding slots cancel in the difference, and a host-side greedy
  chunk assignment balances per-(core,window,chunk) loads, so pad is ~1%.
- Self-loop term dis^2*h handled via a per-core selection matmul on the
  resident table (keeps it out of the Q7 gather stream).
- Window-sum + self-term accumulate in PSUM via two matmuls per 256-col
  piece; layer post-ops run feature-major; layer 2 finishes with a
  [16,128]x[16,40] matmul per 128 nodes + log_softmax, node-major output.
"""

import numpy as np
import ml_dtypes

import concourse.bacc as bacc
import concourse.mybir as mybir
from concourse.tile import TileContext
from concourse.bass_utils import run_bass_kernel_spmd

F32 = mybir.dt.float32
BF16 = mybir.dt.bfloat16
I16 = mybir.dt.int16

N_NODES = 100000
N_FEAT = 500
HID = 16
N_CLS = 40
NC = 8           # cores == dst shards == src windows
SHARD = N_NODES // NC          # 12500
NE = 12544                     # padded window width (98*128)
NK = 7                         # dst chunks per core
DCH = NE // NK                 # 1792 dsts per chunk
PW = 256                       # piece width (PSUM [16, 256])
NPIECE = DCH // PW             # 7
KC = 4                         # feature-dim chunks for phase A (500 = 4*125)
CH = N_FEAT // KC              # 125


def _ceil16(x):
    return (x + 15) // 16 * 16


def _preprocess(x, edge_index):
    src = np.asarray(edge_index[0], dtype=np.int64)
    dst = np.asarray(edge_index[1], dtype=np.int64)
    E = src.size

    deg = np.bincount(dst, minlength=N_NODES) + 1  # incl. self-loop
    dis = (1.0 / np.sqrt(deg.astype(np.float64))).astype(np.float32)

    c_of = dst // SHARD
    w_of = src // SHARD

    # Balance per-(core, window, chunk) edge counts (they set KBAR, the padded
    # gather length) by assigning each dst node to a chunk greedily (LPT on
    # the max per-window load, capacity DCH per chunk). The table column
    # order becomes this permuted order; all index/aux arrays follow it.
    cntdw = np.bincount(dst * NC + w_of, minlength=N_NODES * NC).reshape(
        N_NODES, NC)
    tot = cntdw.sum(1)
    pos_all = np.empty(N_NODES, dtype=np.int64)  # node -> table column
    inv_pos = np.full((NC, NE), -1, dtype=np.int64)  # table column -> local
    for c in range(NC):
        base = c * SHARD
        v = cntdw[base:base + SHARD]
        order = np.argsort(-tot[base:base + SHARD], kind="stable")
        sums = np.zeros((NK, NC), dtype=np.int64)
        counts = np.zeros(NK, dtype=np.int64)
        assign = np.empty(SHARD, dtype=np.int64)
        for d in order:
            cand = (sums + v[d]).max(axis=1)
            cand[counts >= DCH] = 1 << 40
            k = int(np.argmin(cand))
            assign[d] = k
            sums[k] += v[d]
            counts[k] += 1
        fill = np.zeros(NK, dtype=np.int64)
        for d in range(SHARD):
            k = assign[d]
            p = k * DCH + fill[k]
            fill[k] += 1
            pos_all[base + d] = p
            inv_pos[c, p] = d
    pd = pos_all[dst]          # dst table position within its core
    ps = pos_all[src]          # src table position within its window
    sl = ps.astype(np.int16)
    k_of = pd // DCH
    dk = pd - k_of * DCH

    NCELL = NC * NC * NK
    cell = (c_of * NC + w_of) * NK + k_of
    order = np.argsort(cell * DCH + dk, kind="stable")
    sl_s = sl[order]
    cell_s = cell[order]

    cellcnt = np.bincount(cell, minlength=NCELL)
    starts = np.concatenate([[0], np.cumsum(cellcnt)])[:-1]
    rank = np.arange(E, dtype=np.int64) - starts[cell_s]

    # +1 for the leading dummy col; multiple of 32 so every idx slice of
    # KW=KBAR/16 int16 columns starts 4B-aligned (the Q7 ucode reads idxs
    # as 32-bit pairs)
    KBAR = (int(cellcnt.max()) + 1 + 31) // 32 * 32
    KW = KBAR // 16

    flat = np.zeros(NCELL * KBAR, dtype=np.int16)
    flat[cell_s * KBAR + 1 + rank] = sl_s
    lists = flat.reshape(NC, NC, NK, KBAR)  # [c, w, k, KBAR]

    # per-(dst, w) counts in table-position order -> per-chunk inclusive
    # cumsum = end positions
    ends = np.zeros((NC, NK, DCH, NC), dtype=np.int64)
    for c in range(NC):
        blk = np.zeros((NE, NC), dtype=np.int64)
        valid = inv_pos[c] >= 0
        blk[valid] = cntdw[c * SHARD + inv_pos[c, valid]]
        for k in range(NK):
            ends[c, k] = np.cumsum(blk[k * DCH:(k + 1) * DCH], axis=0)
    assert ends.max() <= KBAR - 1
    ends = ends.astype(np.int16)

    # wrap idx lists: [c] -> [128, NK*KW]; rows 16w+p; cols k*KW+s = list[s*16+p]
    idxm = np.zeros((NC, 128, NK * KW), dtype=np.int16)
    idxe = np.zeros((NC, 128, NK * (DCH // 16)), dtype=np.int16)
    EW = DCH // 16
    for c in range(NC):
        for w in range(NC):
            for k in range(NK):
                idxm[c, 16 * w:16 * w + 16, k * KW:(k + 1) * KW] = \
                    lists[c, w, k].reshape(KW, 16).T
                idxe[c, 16 * w:16 * w + 16, k * EW:(k + 1) * EW] = \
                    ends[c, k, :, w].reshape(EW, 16).T

    # xT per core in table-position column order, reshaped so one DMA loads a
    # whole [125, KC, DCH] chunk: layout [CH, NK, KC, DCH]
    xT = np.zeros((NC, CH, NK * KC * DCH), dtype=ml_dtypes.bfloat16)
    dis16 = np.zeros((NC, 16, NE), dtype=np.float32)
    for c in range(NC):
        valid = inv_pos[c] >= 0
        xc = np.zeros((NE, N_FEAT), dtype=np.float32)
        xc[valid] = x[c * SHARD + inv_pos[c, valid]]
        xt = np.ascontiguousarray(xc.T)  # [500, NE]
        xr = xt.reshape(KC, CH, NK, DCH).transpose(1, 2, 0, 3)
        xT[c] = xr.reshape(CH, NK * KC * DCH).astype(ml_dtypes.bfloat16)
        drow = np.zeros(NE, dtype=np.float32)
        drow[valid] = dis[c * SHARD + inv_pos[c, valid]]
        dis16[c] = drow[None, :]

    sel16 = np.zeros((128, HID), dtype=np.float32)
    for w in range(NC):
        sel16[16 * w + np.arange(HID), np.arange(HID)] = 1.0
    selc = np.zeros((NC, 128, HID), dtype=np.float32)
    for c in range(NC):
        selc[c, 16 * c + np.arange(HID), np.arange(HID)] = 1.0

    return dict(idxm=idxm, idxe=idxe, xT=xT, dis16=dis16, sel16=sel16,
                selc=selc, KBAR=KBAR, KW=KW, inv_pos=inv_pos)


def _build_kernel(KBAR, KW):
    EW = DCH // 16
    nc = bacc.Bacc("TRN2")

    xT_d = nc.dram_tensor("xT", [CH, NK * KC * DCH], BF16,
                          kind="ExternalInput")
    w1_d = nc.dram_tensor("w1", [N_FEAT, HID], BF16, kind="ExternalInput")
    w2_d = nc.dram_tensor("w2", [HID, N_CLS], F32, kind="ExternalInput")
    b1_d = nc.dram_tensor("b1c", [HID, 1], F32, kind="ExternalInput")
    b2_d = nc.dram_tensor("b2r", [128, N_CLS], F32, kind="ExternalInput")
    dis_d = nc.dram_tensor("dis16", [HID, NE], F32, kind="ExternalInput")
    s16_d = nc.dram_tensor("sel16", [128, HID], F32, kind="ExternalInput")
    selc_d = nc.dram_tensor("selc", [128, HID], F32, kind="ExternalInput")
    idxm_d = nc.dram_tensor("idxm", [128, NK * KW], I16, kind="ExternalInput")
    idxe_d = nc.dram_tensor("idxe", [128, NK * EW], I16, kind="ExternalInput")
    out_d = nc.dram_tensor("out", [NE, N_CLS], F32, kind="ExternalOutput")

    h1_own = nc.dram_tensor("h1_own", [HID, NE], F32)
    y2_own = nc.dram_tensor("y2_own", [HID, NE], F32)
    import os
    DBG = os.environ.get("GCN_DEBUG") == "1"
    if DBG:
        dbg1_d = nc.dram_tensor("dbg1", [128, NE], F32, kind="ExternalOutput")
        dbg2_d = nc.dram_tensor("dbg2", [HID, NE], F32, kind="ExternalOutput")
    table1 = nc.dram_tensor("table1", [128, NE], F32, addr_space="Shared")
    table2 = nc.dram_tensor("table2", [128, NE], F32, addr_space="Shared")
    groups = [list(range(NC))]

    with TileContext(nc) as tc:
        with tc.tile_pool(name="const", bufs=1) as constp, \
             tc.tile_pool(name="xs", bufs=2) as xsp, \
             tc.tile_pool(name="tab", bufs=1) as tabp, \
             tc.tile_pool(name="g", bufs=2) as gpool, \
             tc.tile_pool(name="eg", bufs=2) as epool, \
             tc.tile_pool(name="dd", bufs=2) as dpool, \
             tc.tile_pool(name="dis", bufs=4) as disp, \
             tc.tile_pool(name="pre", bufs=4) as prep, \
             tc.tile_pool(name="sm", bufs=6) as smp, \
             tc.tile_pool(name="psA", bufs=2, space="PSUM") as psA, \
             tc.tile_pool(name="psL", bufs=3, space="PSUM") as psL, \
             tc.tile_pool(name="psO", bufs=2, space="PSUM") as psO:

            w1t = constp.tile([CH, KC, HID], BF16)
            for k in range(KC):
                nc.sync.dma_start(out=w1t[:, k, :],
                                  in_=w1_d[k * CH:(k + 1) * CH, :])
            w2t = constp.tile([HID, N_CLS], F32)
            nc.sync.dma_start(out=w2t[:], in_=w2_d[:])
            b1t = constp.tile([HID, 1], F32)
            nc.sync.dma_start(out=b1t[:], in_=b1_d[:])
            b2t = constp.tile([128, N_CLS], F32)
            nc.sync.dma_start(out=b2t[:], in_=b2_d[:])
            s16 = constp.tile([128, HID], F32)
            nc.sync.dma_start(out=s16[:], in_=s16_d[:])
            sct = constp.tile([128, HID], F32)
            nc.sync.dma_start(out=sct[:], in_=selc_d[:])
            idxm_t = constp.tile([128, NK * KW], I16)
            nc.sync.dma_start(out=idxm_t[:], in_=idxm_d[:])
            idxe_t = constp.tile([128, NK * EW], I16)
            nc.sync.dma_start(out=idxe_t[:], in_=idxe_d[:])

            # ---------------- Phase A: h1_own = dis * (x @ W1), feature-major
            for j7 in range(NK):
                c0 = j7 * DCH
                xs = xsp.tile([CH, KC, DCH], BF16, tag="xs")
                nc.sync.dma_start(
                    out=xs[:],
                    in_=xT_d[:, j7 * KC * DCH:(j7 + 1) * KC * DCH])
                PWA = 448
                for jp in range(DCH // PWA):
                    j = jp * PWA
                    ps = psA.tile([HID, PWA], F32, tag="psA")
                    for k in range(KC):
                        nc.tensor.matmul(
                            out=ps[:], lhsT=w1t[:, k, :],
                            rhs=xs[:, k, j:j + PWA],
                            start=(k == 0), stop=(k == KC - 1))
                    dsl = disp.tile([HID, PWA], F32, tag="disA")
                    nc.sync.dma_start(out=dsl[:],
                                      in_=dis_d[:, c0 + j:c0 + j + PWA])
                    hsb = prep.tile([HID, PWA], F32, tag="preA")
                    nc.vector.tensor_mul(out=hsb[:], in0=ps[:], in1=dsl[:])
                    nc.sync.dma_start(out=h1_own[:, c0 + j:c0 + j + PWA],
                                      in_=hsb[:])

            nc.gpsimd.collective_compute(
                "AllGather", mybir.AluOpType.bypass, replica_groups=groups,
                ins=[h1_own[:, :]], outs=[table1[:, :]])

            def layer(table_dram, post_piece):
                tab = tabp.tile([128, NE, 1], F32, tag="tab")
                nc.sync.dma_start(out=tab[:, :, 0], in_=table_dram[:, :])

                def main_gather(k):
                    g = gpool.tile([128, KBAR, 1], F32, tag="g")
                    nc.gpsimd.ap_gather(
                        out_ap=g[:], in_ap=tab[:],
                        idxs_ap=idxm_t[:, k * KW:(k + 1) * KW],
                        channels=128, num_elems=NE, d=1, num_idxs=KBAR)
                    return g

                g_prev = main_gather(0)
                for k in range(NK):
                    g = g_prev
                    if k + 1 < NK:
                        g_prev = main_gather(k + 1)
                    # in-place cumsum along slots
                    nc.vector.tensor_tensor_scan(
                        out=g[:, :, 0], data0=g[:, :, 0], data1=g[:, :, 0],
                        initial=0.0, op0=mybir.AluOpType.add,
                        op1=mybir.AluOpType.bypass)
                    eg = epool.tile([128, DCH, 1], F32, tag="eg")
                    nc.gpsimd.ap_gather(
                        out_ap=eg[:], in_ap=g[:],
                        idxs_ap=idxe_t[:, k * EW:(k + 1) * EW],
                        channels=128, num_elems=KBAR, d=1, num_idxs=DCH)
                    dt = dpool.tile([128, DCH], F32, tag="dd")
                    nc.vector.tensor_sub(out=dt[:, 1:], in0=eg[:, 1:, 0],
                                         in1=eg[:, :DCH - 1, 0])
                    nc.vector.tensor_sub(out=dt[:, 0:1], in0=eg[:, 0:1, 0],
                                         in1=g[:, 0:1, 0])
                    for jp in range(NPIECE):
                        j = jp * PW
                        col = k * DCH + j
                        ps = psL.tile([HID, PW], F32, tag="psL")
                        nc.tensor.matmul(out=ps[:], lhsT=s16[:],
                                         rhs=dt[:, j:j + PW],
                                         start=True, stop=False)
                        nc.tensor.matmul(out=ps[:], lhsT=sct[:],
                                         rhs=tab[:, col:col + PW, 0],
                                         start=False, stop=True)
                        dsl = disp.tile([HID, PW], F32, tag="dis")
                        nc.sync.dma_start(out=dsl[:],
                                          in_=dis_d[:, col:col + PW])
                        post_piece(col, ps, dsl)

            # ---------------- Layer 1
            def post1(col, ps, dsl):
                t = prep.tile([HID, PW], F32, tag="pre")
                nc.vector.tensor_mul(out=t[:], in0=ps[:], in1=dsl[:])
                r = prep.tile([HID, PW], F32, tag="pre2")
                nc.scalar.activation(out=r[:], in_=t[:],
                                     func=mybir.ActivationFunctionType.Relu,
                                     bias=b1t[:])
                y = prep.tile([HID, PW], F32, tag="pre3")
                nc.vector.tensor_mul(out=y[:], in0=r[:], in1=dsl[:])
                nc.sync.dma_start(out=y2_own[:, col:col + PW], in_=y[:])

            layer(table1, post1)

            nc.gpsimd.collective_compute(
                "AllGather", mybir.AluOpType.bypass, replica_groups=groups,
                ins=[y2_own[:, :]], outs=[table2[:, :]])

            if DBG:
                nc.sync.dma_start(out=dbg1_d[:, :], in_=table1[:, :])
                nc.sync.dma_start(out=dbg2_d[:, :], in_=y2_own[:, :])

            # ---------------- Layer 2 + output head
            def post2(col, ps, dsl):
                t = prep.tile([HID, PW], F32, tag="pre")
                nc.vector.tensor_mul(out=t[:], in0=ps[:], in1=dsl[:])
                for h in range(PW // 128):
                    po = psO.tile([128, N_CLS], F32, tag="psO")
                    nc.tensor.matmul(out=po[:],
                                     lhsT=t[:, h * 128:(h + 1) * 128],
                                     rhs=w2t[:], start=True, stop=True)
                    o1 = smp.tile([128, N_CLS], F32, tag="o1")
                    nc.vector.tensor_add(out=o1[:], in0=po[:], in1=b2t[:])
                    mx = smp.tile([128, 1], F32, tag="mx")
                    nc.vector.reduce_max(out=mx[:], in_=o1[:],
                                         axis=mybir.AxisListType.X)
                    tt = smp.tile([128, N_CLS], F32, tag="tt")
                    nc.vector.tensor_scalar(out=tt[:], in0=o1[:],
                                            scalar1=mx[:], scalar2=None,
                                            op0=mybir.AluOpType.subtract)
                    ex = smp.tile([128, N_CLS], F32, tag="ex")
                    nc.scalar.activation(
                        out=ex[:], in_=tt[:],
                        func=mybir.ActivationFunctionType.Exp)
                    sm = smp.tile([128, 1], F32, tag="sm")
                    nc.vector.reduce_sum(out=sm[:], in_=ex[:],
                                         axis=mybir.AxisListType.X)
                    ls = smp.tile([128, 1], F32, tag="ls")
                    nc.scalar.activation(
                        out=ls[:], in_=sm[:],
                        func=mybir.ActivationFunctionType.Ln)
                    fin = smp.tile([128, N_CLS], F32, tag="fin")
                    nc.vector.tensor_scalar(out=fin[:], in0=tt[:],
                                            scalar1=ls[:], scalar2=None,
                                            op0=mybir.AluOpType.subtract)
                    rr = col + h * 128
                    nc.sync.dma_start(out=out_d[rr:rr + 128, :], in_=fin[:])

            layer(table2, post2)

    nc.compile()
    return nc


def kernel(x, edge_index, W1, b1, W2, b2):
    x = np.asarray(x, np.float32)
    edge_index = np.asarray(edge_index)
    W1 = np.asarray(W1, np.float32)
    b1 = np.asarray(b1, np.float32)
    W2 = np.asarray(W2, np.float32)
    b2 = np.asarray(b2, np.float32)

    pre = _preprocess(x, edge_index)
    nc = _build_kernel(pre["KBAR"], pre["KW"])

    w1b = W1.astype(ml_dtypes.bfloat16)
    b1c = b1[:, None].copy()
    b2r = np.broadcast_to(b2, (128, N_CLS)).copy()
    in_maps = []
    for c in range(NC):
        in_maps.append({
            "xT": pre["xT"][c],
            "w1": w1b,
            "w2": W2,
            "b1c": b1c,
            "b2r": b2r,
            "dis16": pre["dis16"][c],
            "sel16": pre["sel16"],
            "selc": pre["selc"][c],
            "idxm": pre["idxm"][c],
            "idxe": pre["idxe"][c],
        })
    r = run_bass_kernel_spmd(nc, in_maps, list(range(NC)))
    out = np.empty((N_NODES, N_CLS), dtype=np.float32)
    for c in range(NC):
        inv = pre["inv_pos"][c]
        valid = inv >= 0
        out[c * SHARD + inv[valid]] = r.results[c]["out"][valid]
    return out
